# revision 20
# baseline (speedup 1.0000x reference)
"""Trainium2 Bass kernel for AbnormalitySpecificLoss.

B=256, N=196, D=768, A=14, L=4 hardcoded; data-parallel over the batch dim
across 8 NeuronCores, two SPMD launches:

  Launch A (per core, 32-batch shard), per batch:
    - G = X X^T raw gram in two row chunks (fp32 matmuls, contraction over D).
      The second chunk's lhsT carries q^T as 14 extra stationary columns, so
      scores = q @ x^T rides along for free (psum partitions 96:110).
    - softmax on the scores rows (ACT Exp w/ accumulated Z), attn^T via PE
      transpose (tile_position=(96,0)), pooled = attn @ x (exact fp32).
    - orth partials: Q = G*G (ACT Square), s = 1/diag(G) (eye-mask + reduce +
      reciprocal), out_b = s^T Q s via tiny f32 matmuls.
    - attention-entropy partials: ACT Ln, DVE mult, DVE reduce.
  Host mid: normalize pooled -> pn, transpose to pnT, masks + v = pn^T @ mask.
  Launch B (per core): S_a = Pn_a Pn_a^T rows (f32r), hinge = pos^T relu(S-.3)
  neg via PE row-mask matmul + DVE col-mask dot; pos/neg row sums via v mm.
  Host final: counts, guards, weighted sums (exact reference formulas).
"""

import os

import numpy as np

import concourse.bacc as bacc
import concourse.mybir as mybir
import concourse.tile as tile
from concourse.bass_utils import run_bass_kernel_spmd
from concourse.masks import make_identity

# All activation funcs we use (Exp, Ln, Square, Copy, Relu) live in the
# natural_log_exp_and_others table set.  bacc's greedy per-function set choice
# would thrash between exp_and_others / natural_log every batch (~2.7us per
# reload on the ACT critical path), so empty out every other set.
_orig_gat = bacc.get_activation_tables


def _gat_one_set(arch):
    tabs = _orig_gat(arch)
    return {
        name: (fns if name == "natural_log_exp_and_others" else set())
        for name, fns in tabs.items()
    }


bacc.get_activation_tables = _gat_one_set

F32 = mybir.dt.float32
F32R = mybir.dt.float32r
AF = mybir.ActivationFunctionType
ALU = mybir.AluOpType
AX = mybir.AxisListType

B, N, D, A, L = 256, 196, 768, 14, 4
NCORES = 8
BL = B // NCORES  # 32 batches per core
KD = D // 128  # 6 contraction chunks over D
MARGIN = 0.7
R0 = 100  # gram row-chunk 0 rows; chunk 1 = N-R0 = 96 G rows + 14 score rows
R1 = N - R0  # 96 -> scores land at psum partition 96 (32-aligned for PE)

_CACHE = {}


def _round_f32r(x: np.ndarray) -> np.ndarray:
    """Round-to-nearest to 11 mantissa bits (what the PE's f32r path keeps)."""
    b = np.ascontiguousarray(x, dtype=np.float32).view(np.uint32)
    r = ((b.astype(np.uint64) + 0x800) >> 12 << 12).astype(np.uint32)
    return r.view(np.float32)


def build_kernel_a():
    nc = bacc.Bacc("TRN2", target_bir_lowering=False, debug=False, num_devices=NCORES)
    x_nat = nc.dram_tensor("x_nat", [BL, N, D], F32, kind="ExternalInput").ap()
    # xtq[b, k, p, :] = [ x^T[k*128+p, 0:196] | q^T[k*128+p, 0:14] ]
    xtq_d = nc.dram_tensor("xtq_d", [BL, KD, 128, N + A], F32, kind="ExternalInput").ap()
    attn_w = nc.dram_tensor("attn_w", [L, 128, 9604], F32, kind="ExternalInput").ap()

    pooled_out = nc.dram_tensor("pooled_out", [BL, A, D], F32, kind="ExternalOutput").ap()
    orth_out = nc.dram_tensor("orth_out", [1, BL], F32, kind="ExternalOutput").ap()
    ent_out = nc.dram_tensor("ent_out", [128, 16], F32, kind="ExternalOutput").ap()

    ECH = 2401  # entropy free-dim chunk (4 per layer)

    with tile.TileContext(nc) as tc:
        with (
            tc.tile_pool(name="consts", bufs=1) as consts,
            tc.tile_pool(name="sbuf", bufs=3) as pool,
            tc.tile_pool(name="psum", bufs=1, space="PSUM") as psum,
            tc.tile_pool(name="psum2", bufs=2, space="PSUM") as psum2,
        ):
            # ---- constants ----
            ident = consts.tile([128, 128], F32, tag="ident")
            make_identity(nc, ident[:])
            # eyeT[:, 0:N]: diagonal at 0 (gram chunk 0); [:, N:2N]: diag at +R0
            eyeT = consts.tile([128, 2 * N], F32, tag="eyeT")
            nc.gpsimd.memset(eyeT[:], 0.0)
            nc.gpsimd.affine_select(
                out=eyeT[:, 0:N], in_=eyeT[:, 0:N],
                compare_op=ALU.not_equal, fill=1.0,
                base=0, pattern=[[-1, N]], channel_multiplier=1,
            )
            nc.gpsimd.affine_select(
                out=eyeT[:, N : 2 * N], in_=eyeT[:, N : 2 * N],
                compare_op=ALU.not_equal, fill=1.0,
                base=R0, pattern=[[-1, N]], channel_multiplier=1,
            )
            eps_b = consts.tile([128, 1], F32, tag="eps")
            nc.vector.memset(eps_b[:], 1e-6)
            eacc = consts.tile([128, 16], F32, tag="eacc")
            orth_sb = consts.tile([1, BL], F32, tag="orth_sb")

            # ---- per-batch pipeline ----
            for b in range(BL):
                xtq = pool.tile([128, KD, N + A], F32, tag="xtq")
                nc.sync.dma_start(xtq[:], xtq_d[b].rearrange("k p n -> p k n"))
                xa = pool.tile([128, D], F32, tag="xa")
                nc.sync.dma_start(xa[:], x_nat[b, 0:128, :])
                xb = pool.tile([68, D], F32, tag="xb")
                nc.sync.dma_start(xb[:], x_nat[b, 128:N, :])

                # G row chunks; chunk1 lhsT carries q^T -> scores at rows 96:110
                ps_g0 = psum2.tile([R0, N], F32, tag="ps_g0")
                for k in range(KD):
                    nc.tensor.matmul(
                        ps_g0[:], xtq[:, k, 0:R0], xtq[:, k, 0:N],
                        start=(k == 0), stop=(k == KD - 1),
                    )
                ps_g1 = psum2.tile([R1 + A, N], F32, tag="ps_g1")
                for k in range(KD):
                    nc.tensor.matmul(
                        ps_g1[:], xtq[:, k, R0 : N + A], xtq[:, k, 0:N],
                        start=(k == 0), stop=(k == KD - 1),
                    )

                # softmax on scores rows [96:110] (lane-aligned slices)
                mx = pool.tile([R1 + A, 1], F32, tag="mx")
                nc.vector.reduce_max(mx[R1:, :], ps_g1[R1:, :], axis=AX.X)
                negm = pool.tile([R1 + A, 1], F32, tag="negm")
                nc.vector.tensor_scalar_mul(negm[R1:, :], mx[R1:, :], -1.0)
                expt = pool.tile([R1 + A, N], F32, tag="expt")
                zsum = pool.tile([R1 + A, 1], F32, tag="zsum")
                nc.scalar.activation(
                    expt[R1:, :], ps_g1[R1:, :], AF.Exp,
                    bias=negm[R1:, :], scale=1.0, accum_out=zsum[R1:, :],
                )
                rz = pool.tile([R1 + A, 1], F32, tag="rz")
                nc.vector.reciprocal(rz[R1:, :], zsum[R1:, :])
                attn = pool.tile([R1 + A, N], F32, tag="attn")
                nc.vector.tensor_scalar_mul(attn[R1:, :], expt[R1:, :], rz[R1:, :])

                # attn^T via PE transpose (lhsT at base partition 96)
                ps_at = psum.tile([128, 2 * A], F32, tag="ps_at")
                nc.tensor.transpose(
                    ps_at[:, 0:A], attn[R1:, 0:128],
                    ident[R1 : R1 + A, R1 : R1 + A], tile_position=(R1, 0),
                )
                nc.tensor.transpose(
                    ps_at[0:68, A : 2 * A], attn[R1:, 128:N],
                    ident[R1 : R1 + A, R1 : R1 + A], tile_position=(R1, 0),
                )
                attnT = pool.tile([128, 2 * A], F32, tag="attnT")
                nc.vector.tensor_copy(attnT[:, 0:A], ps_at[:, 0:A])
                nc.vector.tensor_copy(attnT[0:68, A : 2 * A], ps_at[0:68, A : 2 * A])

                # pooled = attn @ x -> psum [A, D] (exact fp32)
                ps_pool = psum.tile([A, D], F32, tag="ps_pool")
                for f0, f1 in ((0, 512), (512, D)):
                    nc.tensor.matmul(
                        ps_pool[:, f0:f1], attnT[:, 0:A], xa[:, f0:f1],
                        start=True, stop=False,
                    )
                    nc.tensor.matmul(
                        ps_pool[:, f0:f1], attnT[0:68, A : 2 * A], xb[:, f0:f1],
                        start=False, stop=True,
                    )
                pooled_sb = pool.tile([A, D], F32, tag="pooled_sb")
                nc.scalar.copy(pooled_sb[:], ps_pool[:])
                nc.sync.dma_start(pooled_out[b], pooled_sb[:])

                # Q = G*G via ACT Square (evacuates PSUM); diag via eye-mask
                q0 = pool.tile([R0, N], F32, tag="q0")
                nc.scalar.activation(q0[:], ps_g0[:], AF.Square)
                q1 = pool.tile([R1, N], F32, tag="q1")
                nc.scalar.activation(q1[:], ps_g1[0:R1, :], AF.Square)
                dt0 = pool.tile([R0, N], F32, tag="dt0")
                nc.vector.tensor_tensor(
                    out=dt0[:], in0=ps_g0[:], in1=eyeT[0:R0, 0:N], op=ALU.mult
                )
                dt1 = pool.tile([R1, N], F32, tag="dt1")
                nc.vector.tensor_tensor(
                    out=dt1[:], in0=ps_g1[0:R1, :], in1=eyeT[0:R1, N : 2 * N],
                    op=ALU.mult,
                )
                dg0 = pool.tile([R0, 1], F32, tag="dg0")
                nc.vector.reduce_sum(dg0[:], dt0[:], axis=AX.X)
                dg1 = pool.tile([R1, 1], F32, tag="dg1")
                nc.vector.reduce_sum(dg1[:], dt1[:], axis=AX.X)
                s0 = pool.tile([R0, 1], F32, tag="s0")
                nc.vector.reciprocal(s0[:], dg0[:])
                s1 = pool.tile([R1, 1], F32, tag="s1")
                nc.vector.reciprocal(s1[:], dg1[:])

                # u = Q^T s (chunked), then dot = u^T s  (plain f32 matmuls)
                ps_u = psum.tile([128, 4], F32, tag="ps_u")
                nc.tensor.matmul(ps_u[0:R0, 0:1], q0[:, 0:R0], s0[:], start=True, stop=False)
                nc.tensor.matmul(ps_u[0:R0, 0:1], q1[:, 0:R0], s1[:], start=False, stop=True)
                nc.tensor.matmul(ps_u[0:R1, 1:2], q0[:, R0:N], s0[:], start=True, stop=False)
                nc.tensor.matmul(ps_u[0:R1, 1:2], q1[:, R0:N], s1[:], start=False, stop=True)
                u0 = pool.tile([R0, 1], F32, tag="u0")
                nc.vector.tensor_copy(u0[:], ps_u[0:R0, 0:1])
                u1 = pool.tile([R1, 1], F32, tag="u1")
                nc.vector.tensor_copy(u1[:], ps_u[0:R1, 1:2])
                nc.tensor.matmul(ps_u[0:1, 2:3], u0[:], s0[:], start=True, stop=False)
                nc.tensor.matmul(ps_u[0:1, 2:3], u1[:], s1[:], start=False, stop=True)
                nc.vector.tensor_copy(orth_sb[0:1, b : b + 1], ps_u[0:1, 2:3])

                # entropy chunk (one per two batches)
                if b % 2 == 0:
                    c = b // 2
                    lay, sub = c // 4, c % 4
                    wt = pool.tile([128, ECH], F32, tag="wt")
                    nc.sync.dma_start(
                        wt[:], attn_w[lay, :, sub * ECH : (sub + 1) * ECH]
                    )
                    lnt = pool.tile([128, ECH], F32, tag="lnt")
                    nc.scalar.activation(lnt[:], wt[:], AF.Ln, bias=eps_b[:], scale=1.0)
                    prod = pool.tile([128, ECH], F32, tag="prod")
                    nc.vector.tensor_tensor(out=prod[:], in0=wt[:], in1=lnt[:], op=ALU.mult)
                    dump = pool.tile([128, ECH], F32, tag="dump")
                    nc.scalar.activation(
                        dump[:], prod[:], AF.Copy, accum_out=eacc[:, c : c + 1]
                    )

            nc.sync.dma_start(orth_out[:], orth_sb[:])
            nc.sync.dma_start(ent_out[:], eacc[:])
    nc.compile()
    return nc


def build_kernel_b():
    nc = bacc.Bacc("TRN2", target_bir_lowering=False, debug=False, num_devices=NCORES)
    # replicated: pnT [A, D, B] (f32r), v masks [2, A, KD, 128] (f32r), negm rows
    pnt = nc.dram_tensor("pnt", [A, KD, 128, B], F32R, kind="ExternalInput").ap()
    # per-core: local columns slice [A, KD, 128, BL], local pos mask [BL, A]
    pntl = nc.dram_tensor("pntl", [A, KD, 128, BL], F32R, kind="ExternalInput").ap()
    vmask = nc.dram_tensor("vmask", [2, A, KD, 128], F32R, kind="ExternalInput").ap()
    posml = nc.dram_tensor("posml", [BL, A], F32, kind="ExternalInput").ap()
    negm = nc.dram_tensor("negm", [1, A, B], F32, kind="ExternalInput").ap()

    # sums[0, a, 0:B?]: possum/negsum per (a, local b) plus hinge value
    sums_out = nc.dram_tensor("sums_out", [1, A, 2 * BL + 1], F32, kind="ExternalOutput").ap()

    with tile.TileContext(nc) as tc:
        with (
            tc.tile_pool(name="consts", bufs=1) as consts,
            tc.tile_pool(name="sbuf", bufs=4) as pool,
            tc.tile_pool(name="psum", bufs=2, space="PSUM") as psum,
        ):
            posm_sb = consts.tile([BL, A], F32, tag="posm")
            nc.sync.dma_start(posm_sb[:], posml[:])
            negm_sb = consts.tile([1, A, B], F32, tag="negm")
            nc.sync.dma_start(negm_sb[:], negm[:])
            vm_sb = consts.tile([128, 2, A, KD], F32R, tag="vm")
            nc.sync.dma_start(vm_sb[:], vmask.rearrange("m a k p -> p m a k"))
            mb = consts.tile([BL, 1], F32, tag="mb")
            nc.vector.memset(mb[:], MARGIN - 1.0)
            sums_sb = consts.tile([1, A, 2 * BL + 1], F32, tag="sums")

            for a in range(A):
                pt = pool.tile([128, KD, B], F32R, tag="pt")
                nc.sync.dma_start(pt[:], pnt[a].rearrange("k p c -> p k c"))
                ptl = pool.tile([128, KD, BL], F32R, tag="ptl")
                nc.sync.dma_start(ptl[:], pntl[a].rearrange("k p c -> p k c"))

                # S rows: [BL, B]
                ps_s = psum.tile([BL, B], F32, tag="ps_s")
                for k in range(KD):
                    nc.tensor.matmul(
                        ps_s[:], ptl[:, k, :], pt[:, k, :],
                        start=(k == 0), stop=(k == KD - 1),
                    )
                # possum/negsum: v^T @ pnT_local -> [1, BL] each
                ps_vs = psum.tile([1, 2 * BL], F32, tag="ps_vs")
                for m in range(2):
                    for k in range(KD):
                        nc.tensor.matmul(
                            ps_vs[:, m * BL : (m + 1) * BL],
                            vm_sb[:, m, a, k : k + 1],
                            ptl[:, k, :],
                            start=(k == 0), stop=(k == KD - 1),
                        )
                nc.vector.tensor_copy(sums_sb[0:1, a, 0 : 2 * BL], ps_vs[:])

                # hinge: H = relu(S - 0.3); col = posm^T @ H; dot with negm
                ht = pool.tile([BL, B], F32, tag="ht")
                nc.scalar.activation(ht[:], ps_s[:], AF.Relu, bias=mb[:], scale=1.0)
                ps_h = psum.tile([1, B], F32, tag="ps_h")
                nc.tensor.matmul(
                    ps_h[:], posm_sb[:, a : a + 1], ht[:], start=True, stop=True
                )
                hrow = pool.tile([1, B], F32, tag="hrow")
                nc.vector.tensor_tensor(
                    out=hrow[:], in0=ps_h[:], in1=negm_sb[0:1, a, :], op=ALU.mult
                )
                nc.vector.reduce_sum(
                    sums_sb[0:1, a, 2 * BL : 2 * BL + 1], hrow[:], axis=AX.X
                )

            nc.sync.dma_start(sums_out[:], sums_sb[:])
    nc.compile()
    return nc


def _prep_a_inputs(x, attn, q):
    """Per-core in_maps for launch A."""
    xt_full = np.ascontiguousarray(x.transpose(0, 2, 1))  # [B, D, N]
    q_t = np.ascontiguousarray(q.T).reshape(KD, 128, A)
    in_maps = []
    for c in range(NCORES):
        sl = slice(c * BL, (c + 1) * BL)
        x_nat = np.ascontiguousarray(x[sl])
        xtq = np.empty((BL, KD, 128, N + A), np.float32)
        xtq[:, :, :, 0:N] = xt_full[sl].reshape(BL, KD, 128, N)
        xtq[:, :, :, N:] = q_t[None]
        aw = np.ascontiguousarray(attn[:, sl]).reshape(L, 128, 9604)
        in_maps.append({"x_nat": x_nat, "xtq_d": xtq, "attn_w": aw})
    return in_maps


def _prep_b_inputs(pn, pos_m, neg_m):
    """pn [B, A, D] normalized pooled; masks [A, B] f32."""
    pnt = _round_f32r(
        np.ascontiguousarray(pn.transpose(1, 2, 0)).reshape(A, KD, 128, B)
    )
    # v = sum_c pn[c,a,:] * mask[a,c] -> [2, A, D]
    v = np.stack(
        [
            np.einsum("cad,ac->ad", pn.astype(np.float64), m.astype(np.float64))
            for m in (pos_m, neg_m)
        ]
    ).astype(np.float32)
    vmask = _round_f32r(v.reshape(2, A, KD, 128))
    negm = np.ascontiguousarray(neg_m, dtype=np.float32).reshape(1, A, B)
    in_maps = []
    for c in range(NCORES):
        sl = slice(c * BL, (c + 1) * BL)
        pntl = np.ascontiguousarray(pnt[:, :, :, sl])
        posml = np.ascontiguousarray(pos_m.T[sl]).astype(np.float32)
        in_maps.append(
            {"pnt": pnt, "pntl": pntl, "vmask": vmask, "posml": posml, "negm": negm}
        )
    return in_maps


def kernel(common_representations, attention_weights, query_vectors, labels):
    x = np.asarray(common_representations, dtype=np.float32)
    attn = np.asarray(attention_weights, dtype=np.float32)
    q = np.asarray(query_vectors, dtype=np.float32)
    labels = np.asarray(labels)

    if "a" not in _CACHE:
        _CACHE["a"] = build_kernel_a()
    if "b" not in _CACHE:
        _CACHE["b"] = build_kernel_b()

    core_ids = list(range(NCORES))
    profile = os.environ.get("BASS_KERNEL_PROFILE", "0") == "1"
    if profile:
        _CACHE["profile"] = []
    ra = run_bass_kernel_spmd(
        _CACHE["a"], _prep_a_inputs(x, attn, q), core_ids, trace=profile
    )
    if profile:
        _CACHE["profile"].append(ra.exec_time_ns)
    res_a = ra.results

    pooled = np.concatenate([r["pooled_out"] for r in res_a], axis=0)  # [B, A, D]
    orth_rows = np.concatenate([r["orth_out"][0] for r in res_a])  # [B]
    ent = np.stack([r["ent_out"] for r in res_a])  # [NCORES, 128, 16]

    # --- host: orth + sparsity scalars ---
    orth_loss = (orth_rows.astype(np.float64).sum() - B * N) / (B * N * N)

    ent_chunks = ent.astype(np.float64).sum(axis=(0, 1))  # [16]
    per_layer = -ent_chunks.reshape(L, 4).sum(axis=1) / (B * N * N)
    lin = np.linspace(-2.0, 2.0, L)
    layer_w = 1.0 / (1.0 + np.exp(-lin))
    sparsity_loss = (layer_w * per_layer).sum() / L

    # --- host: pn + masks, then launch B ---
    pn64 = pooled.astype(np.float64)
    pn = (pn64 / np.linalg.norm(pn64, axis=-1, keepdims=True)).astype(np.float32)
    lt = labels.T  # [A, B]
    pos_m = (lt == 1).astype(np.float32)
    neg_m = (lt == 0).astype(np.float32)
    unc_m = (lt == 2).astype(np.float32)

    rb = run_bass_kernel_spmd(
        _CACHE["b"], _prep_b_inputs(pn, pos_m, neg_m), core_ids, trace=profile
    )
    if profile:
        _CACHE["profile"].append(rb.exec_time_ns)
    res_b = rb.results
    sums = np.stack([r["sums_out"][0] for r in res_b])  # [NCORES, A, 2*BL+1]

    possum = np.concatenate([sums[c, :, 0:BL] for c in range(NCORES)], axis=1)  # [A,B]
    negsum = np.concatenate([sums[c, :, BL : 2 * BL] for c in range(NCORES)], axis=1)
    hinge_sum = sums[:, :, 2 * BL].sum(axis=0)  # [A]

    n_pos = pos_m.sum(-1).astype(np.float64)
    n_neg = neg_m.sum(-1).astype(np.float64)
    n_unc = unc_m.sum(-1).astype(np.float64)
    pair_cnt = n_pos * n_neg
    pos_neg_loss = np.where(
        pair_cnt > 0, hinge_sum.astype(np.float64) / np.maximum(pair_cnt, 1.0), 0.0
    )
    pos_mean = possum.astype(np.float64) / np.maximum(n_pos, 1.0)[:, None]
    neg_mean = negsum.astype(np.float64) / np.maximum(n_neg, 1.0)[:, None]
    unc_sum = (np.abs(pos_mean - neg_mean) * unc_m).sum(axis=-1)
    unc_ok = (n_unc > 0) & (n_pos > 0) & (n_neg > 0)
    unc_loss = np.where(unc_ok, unc_sum / np.maximum(n_unc, 1.0), 0.0)
    contrastive_loss = (pos_neg_loss + unc_loss).sum() / A

    return (
        pooled.astype(np.float32),
        np.float32(orth_loss),
        np.float32(contrastive_loss),
        np.float32(sparsity_loss),
    )


# revision 21
# speedup vs baseline: 1.0304x; 1.0304x over previous
"""Trainium2 Bass kernel for AbnormalitySpecificLoss.

B=256, N=196, D=768, A=14, L=4 hardcoded; data-parallel over the batch dim
across 8 NeuronCores, two SPMD launches:

  Launch A (per core, 32-batch shard), per batch:
    - G = X X^T raw gram in two row chunks (fp32 matmuls, contraction over D).
      The second chunk's lhsT carries q^T as 14 extra stationary columns, so
      scores = q @ x^T rides along for free (psum partitions 96:110).
    - softmax on the scores rows (ACT Exp w/ accumulated Z), attn^T via PE
      transpose (tile_position=(96,0)), pooled = attn @ x (exact fp32).
    - orth partials: Q = G*G (ACT Square), s = 1/diag(G) (eye-mask + reduce +
      reciprocal), out_b = s^T Q s via tiny f32 matmuls.
    - attention-entropy partials: ACT Ln, DVE mult, DVE reduce.
  Host mid: normalize pooled -> pn, transpose to pnT, masks + v = pn^T @ mask.
  Launch B (per core): S_a = Pn_a Pn_a^T rows (f32r), hinge = pos^T relu(S-.3)
  neg via PE row-mask matmul + DVE col-mask dot; pos/neg row sums via v mm.
  Host final: counts, guards, weighted sums (exact reference formulas).
"""

import os

import numpy as np

import concourse.bacc as bacc
import concourse.mybir as mybir
import concourse.tile as tile
from concourse.bass_utils import run_bass_kernel_spmd
from concourse.masks import make_identity

# All activation funcs we use (Exp, Ln, Square, Copy, Relu) live in the
# natural_log_exp_and_others table set.  bacc's greedy per-function set choice
# would thrash between exp_and_others / natural_log every batch (~2.7us per
# reload on the ACT critical path), so empty out every other set.
_orig_gat = bacc.get_activation_tables


def _gat_one_set(arch):
    tabs = _orig_gat(arch)
    return {
        name: (fns if name == "natural_log_exp_and_others" else set())
        for name, fns in tabs.items()
    }


bacc.get_activation_tables = _gat_one_set

F32 = mybir.dt.float32
F32R = mybir.dt.float32r
AF = mybir.ActivationFunctionType
ALU = mybir.AluOpType
AX = mybir.AxisListType

B, N, D, A, L = 256, 196, 768, 14, 4
NCORES = 8
BL = B // NCORES  # 32 batches per core
KD = D // 128  # 6 contraction chunks over D
MARGIN = 0.7
R0 = 100  # gram row-chunk 0 rows; chunk 1 = N-R0 = 96 G rows + 14 score rows
R1 = N - R0  # 96 -> scores land at psum partition 96 (32-aligned for PE)

_CACHE = {}


def _round_f32r(x: np.ndarray) -> np.ndarray:
    """Round-to-nearest to 11 mantissa bits (what the PE's f32r path keeps)."""
    b = np.ascontiguousarray(x, dtype=np.float32).view(np.uint32)
    r = ((b.astype(np.uint64) + 0x800) >> 12 << 12).astype(np.uint32)
    return r.view(np.float32)


def build_kernel_a():
    nc = bacc.Bacc("TRN2", target_bir_lowering=False, debug=False, num_devices=NCORES)
    x_nat = nc.dram_tensor("x_nat", [BL, N, D], F32, kind="ExternalInput").ap()
    # xtq[b, k, p, :] = [ x^T[k*128+p, 0:196] | q^T[k*128+p, 0:14] ]
    xtq_d = nc.dram_tensor("xtq_d", [BL, KD, 128, N + A], F32, kind="ExternalInput").ap()
    attn_w = nc.dram_tensor("attn_w", [L, 128, 9604], F32, kind="ExternalInput").ap()

    pooled_out = nc.dram_tensor("pooled_out", [BL, A, D], F32, kind="ExternalOutput").ap()
    orth_out = nc.dram_tensor("orth_out", [1, BL], F32, kind="ExternalOutput").ap()
    ent_out = nc.dram_tensor("ent_out", [128, 16], F32, kind="ExternalOutput").ap()

    ECH = 2401  # entropy free-dim chunk (4 per layer)

    with tile.TileContext(nc) as tc:
        with (
            tc.tile_pool(name="consts", bufs=1) as consts,
            tc.tile_pool(name="sbuf", bufs=3) as pool,
            tc.tile_pool(name="psum", bufs=1, space="PSUM") as psum,
            tc.tile_pool(name="psum2", bufs=2, space="PSUM") as psum2,
        ):
            # ---- constants ----
            ident = consts.tile([128, 128], F32, tag="ident")
            make_identity(nc, ident[:])
            # eyeT[:, 0:N]: diagonal at 0 (gram chunk 0); [:, N:2N]: diag at +R0
            eyeT = consts.tile([128, 2 * N], F32, tag="eyeT")
            nc.gpsimd.memset(eyeT[:], 0.0)
            nc.gpsimd.affine_select(
                out=eyeT[:, 0:N], in_=eyeT[:, 0:N],
                compare_op=ALU.not_equal, fill=1.0,
                base=0, pattern=[[-1, N]], channel_multiplier=1,
            )
            nc.gpsimd.affine_select(
                out=eyeT[:, N : 2 * N], in_=eyeT[:, N : 2 * N],
                compare_op=ALU.not_equal, fill=1.0,
                base=R0, pattern=[[-1, N]], channel_multiplier=1,
            )
            eps_b = consts.tile([128, 1], F32, tag="eps")
            nc.vector.memset(eps_b[:], 1e-6)
            eacc = consts.tile([128, 16], F32, tag="eacc")
            orth_sb = consts.tile([1, BL], F32, tag="orth_sb")

            # ---- per-batch pipeline ----
            for b in range(BL):
                xtq = pool.tile([128, KD, N + A], F32, tag="xtq")
                nc.sync.dma_start(xtq[:], xtq_d[b].rearrange("k p n -> p k n"))
                xa = pool.tile([128, D], F32, tag="xa")
                nc.sync.dma_start(xa[:], x_nat[b, 0:128, :])
                xb = pool.tile([68, D], F32, tag="xb")
                nc.sync.dma_start(xb[:], x_nat[b, 128:N, :])

                # G row chunks; chunk1 lhsT carries q^T -> scores at rows 96:110
                ps_g0 = psum2.tile([R0, N], F32, tag="ps_g0")
                for k in range(KD):
                    nc.tensor.matmul(
                        ps_g0[:], xtq[:, k, 0:R0], xtq[:, k, 0:N],
                        start=(k == 0), stop=(k == KD - 1),
                    )
                ps_g1 = psum2.tile([R1 + A, N], F32, tag="ps_g1")
                for k in range(KD):
                    nc.tensor.matmul(
                        ps_g1[:], xtq[:, k, R0 : N + A], xtq[:, k, 0:N],
                        start=(k == 0), stop=(k == KD - 1),
                    )

                # softmax on scores rows [96:110] (lane-aligned slices)
                mx = pool.tile([R1 + A, 1], F32, tag="mx")
                nc.vector.reduce_max(mx[R1:, :], ps_g1[R1:, :], axis=AX.X)
                negm = pool.tile([R1 + A, 1], F32, tag="negm")
                nc.vector.tensor_scalar_mul(negm[R1:, :], mx[R1:, :], -1.0)
                expt = pool.tile([R1 + A, N], F32, tag="expt")
                zsum = pool.tile([R1 + A, 1], F32, tag="zsum")
                nc.scalar.activation(
                    expt[R1:, :], ps_g1[R1:, :], AF.Exp,
                    bias=negm[R1:, :], scale=1.0, accum_out=zsum[R1:, :],
                )
                rz = pool.tile([R1 + A, 1], F32, tag="rz")
                nc.vector.reciprocal(rz[R1:, :], zsum[R1:, :])
                attn = pool.tile([R1 + A, N], F32, tag="attn")
                nc.vector.tensor_scalar_mul(attn[R1:, :], expt[R1:, :], rz[R1:, :])

                # attn^T via PE transpose (lhsT at base partition 96)
                ps_at = psum.tile([128, 2 * A], F32, tag="ps_at")
                nc.tensor.transpose(
                    ps_at[:, 0:A], attn[R1:, 0:128],
                    ident[R1 : R1 + A, R1 : R1 + A], tile_position=(R1, 0),
                )
                nc.tensor.transpose(
                    ps_at[0:68, A : 2 * A], attn[R1:, 128:N],
                    ident[R1 : R1 + A, R1 : R1 + A], tile_position=(R1, 0),
                )
                attnT = pool.tile([128, 2 * A], F32, tag="attnT")
                nc.vector.tensor_copy(attnT[:, 0:A], ps_at[:, 0:A])
                nc.vector.tensor_copy(attnT[0:68, A : 2 * A], ps_at[0:68, A : 2 * A])

                # pooled = attn @ x -> psum [A, D] (exact fp32)
                ps_pool = psum.tile([A, D], F32, tag="ps_pool")
                for f0, f1 in ((0, 512), (512, D)):
                    nc.tensor.matmul(
                        ps_pool[:, f0:f1], attnT[:, 0:A], xa[:, f0:f1],
                        start=True, stop=False,
                    )
                    nc.tensor.matmul(
                        ps_pool[:, f0:f1], attnT[0:68, A : 2 * A], xb[:, f0:f1],
                        start=False, stop=True,
                    )
                pooled_sb = pool.tile([A, D], F32, tag="pooled_sb")
                nc.scalar.copy(pooled_sb[:], ps_pool[:])
                nc.sync.dma_start(pooled_out[b], pooled_sb[:])

                # Q = G*G via ACT Square (evacuates PSUM); diag via eye-mask
                q0 = pool.tile([R0, N], F32, tag="q0")
                nc.scalar.activation(q0[:], ps_g0[:], AF.Square)
                q1 = pool.tile([R1, N], F32, tag="q1")
                nc.scalar.activation(q1[:], ps_g1[0:R1, :], AF.Square)
                dt0 = pool.tile([R0, N], F32, tag="dt0")
                nc.vector.tensor_tensor(
                    out=dt0[:], in0=ps_g0[:], in1=eyeT[0:R0, 0:N], op=ALU.mult
                )
                dt1 = pool.tile([R1, N], F32, tag="dt1")
                nc.vector.tensor_tensor(
                    out=dt1[:], in0=ps_g1[0:R1, :], in1=eyeT[0:R1, N : 2 * N],
                    op=ALU.mult,
                )
                dg0 = pool.tile([R0, 1], F32, tag="dg0")
                nc.vector.reduce_sum(dg0[:], dt0[:], axis=AX.X)
                dg1 = pool.tile([R1, 1], F32, tag="dg1")
                nc.vector.reduce_sum(dg1[:], dt1[:], axis=AX.X)
                s0 = pool.tile([R0, 1], F32, tag="s0")
                nc.vector.reciprocal(s0[:], dg0[:])
                s1 = pool.tile([R1, 1], F32, tag="s1")
                nc.vector.reciprocal(s1[:], dg1[:])

                # u = Q^T s (chunked), then dot = u^T s  (plain f32 matmuls)
                ps_u = psum.tile([128, 4], F32, tag="ps_u")
                nc.tensor.matmul(ps_u[0:R0, 0:1], q0[:, 0:R0], s0[:], start=True, stop=False)
                nc.tensor.matmul(ps_u[0:R0, 0:1], q1[:, 0:R0], s1[:], start=False, stop=True)
                nc.tensor.matmul(ps_u[0:R1, 1:2], q0[:, R0:N], s0[:], start=True, stop=False)
                nc.tensor.matmul(ps_u[0:R1, 1:2], q1[:, R0:N], s1[:], start=False, stop=True)
                u0 = pool.tile([R0, 1], F32, tag="u0")
                nc.vector.tensor_copy(u0[:], ps_u[0:R0, 0:1])
                u1 = pool.tile([R1, 1], F32, tag="u1")
                nc.vector.tensor_copy(u1[:], ps_u[0:R1, 1:2])
                nc.tensor.matmul(ps_u[0:1, 2:3], u0[:], s0[:], start=True, stop=False)
                nc.tensor.matmul(ps_u[0:1, 2:3], u1[:], s1[:], start=False, stop=True)
                nc.vector.tensor_copy(orth_sb[0:1, b : b + 1], ps_u[0:1, 2:3])

                # entropy chunk (one per two batches)
                if b % 2 == 0:
                    c = b // 2
                    lay, sub = c // 4, c % 4
                    wt = pool.tile([128, ECH], F32, tag="wt")
                    nc.sync.dma_start(
                        wt[:], attn_w[lay, :, sub * ECH : (sub + 1) * ECH]
                    )
                    lnt = pool.tile([128, ECH], F32, tag="lnt")
                    nc.scalar.activation(lnt[:], wt[:], AF.Ln, bias=eps_b[:], scale=1.0)
                    prod = pool.tile([128, ECH], F32, tag="prod")
                    nc.vector.tensor_tensor(out=prod[:], in0=wt[:], in1=lnt[:], op=ALU.mult)
                    dump = pool.tile([128, ECH], F32, tag="dump")
                    nc.scalar.activation(
                        dump[:], prod[:], AF.Copy, accum_out=eacc[:, c : c + 1]
                    )

            nc.sync.dma_start(orth_out[:], orth_sb[:])
            nc.sync.dma_start(ent_out[:], eacc[:])
    nc.compile()
    return nc


def build_kernel_b():
    nc = bacc.Bacc("TRN2", target_bir_lowering=False, debug=False, num_devices=NCORES)
    # replicated: pnT [A, D, B] (f32r), v masks [2, A, KD, 128] (f32r), negm rows
    pnt = nc.dram_tensor("pnt", [A, KD, 128, B], F32R, kind="ExternalInput").ap()
    # per-core: local columns slice [A, KD, 128, BL], local pos mask [BL, A]
    pntl = nc.dram_tensor("pntl", [A, KD, 128, BL], F32R, kind="ExternalInput").ap()
    vmask = nc.dram_tensor("vmask", [128, 2, A, KD], F32R, kind="ExternalInput").ap()
    posml = nc.dram_tensor("posml", [BL, A], F32, kind="ExternalInput").ap()
    negm = nc.dram_tensor("negm", [1, A, B], F32, kind="ExternalInput").ap()

    # sums[0, a, 0:B?]: possum/negsum per (a, local b) plus hinge value
    sums_out = nc.dram_tensor("sums_out", [1, A, 2 * BL + 1], F32, kind="ExternalOutput").ap()

    with tile.TileContext(nc) as tc:
        with (
            tc.tile_pool(name="consts", bufs=1) as consts,
            tc.tile_pool(name="sbuf", bufs=4) as pool,
            tc.tile_pool(name="psum", bufs=2, space="PSUM") as psum,
        ):
            posm_sb = consts.tile([BL, A], F32, tag="posm")
            nc.sync.dma_start(posm_sb[:], posml[:])
            negm_sb = consts.tile([1, A, B], F32, tag="negm")
            nc.sync.dma_start(negm_sb[:], negm[:])
            vm_sb = consts.tile([128, 2, A, KD], F32R, tag="vm")
            nc.sync.dma_start(vm_sb[:], vmask[:])
            mb = consts.tile([BL, 1], F32, tag="mb")
            nc.vector.memset(mb[:], MARGIN - 1.0)
            sums_sb = consts.tile([1, A, 2 * BL + 1], F32, tag="sums")

            for a in range(A):
                pt = pool.tile([128, KD, B], F32R, tag="pt")
                nc.sync.dma_start(pt[:], pnt[a].rearrange("k p c -> p k c"))
                ptl = pool.tile([128, KD, BL], F32R, tag="ptl")
                nc.sync.dma_start(ptl[:], pntl[a].rearrange("k p c -> p k c"))

                # S rows: [BL, B]
                ps_s = psum.tile([BL, B], F32, tag="ps_s")
                for k in range(KD):
                    nc.tensor.matmul(
                        ps_s[:], ptl[:, k, :], pt[:, k, :],
                        start=(k == 0), stop=(k == KD - 1),
                    )
                # possum/negsum: v^T @ pnT_local -> [1, BL] each
                ps_vs = psum.tile([1, 2 * BL], F32, tag="ps_vs")
                for m in range(2):
                    for k in range(KD):
                        nc.tensor.matmul(
                            ps_vs[:, m * BL : (m + 1) * BL],
                            vm_sb[:, m, a, k : k + 1],
                            ptl[:, k, :],
                            start=(k == 0), stop=(k == KD - 1),
                        )
                nc.vector.tensor_copy(sums_sb[0:1, a, 0 : 2 * BL], ps_vs[:])

                # hinge: H = relu(S - 0.3); col = posm^T @ H; dot with negm
                ht = pool.tile([BL, B], F32, tag="ht")
                nc.scalar.activation(ht[:], ps_s[:], AF.Relu, bias=mb[:], scale=1.0)
                ps_h = psum.tile([1, B], F32, tag="ps_h")
                nc.tensor.matmul(
                    ps_h[:], posm_sb[:, a : a + 1], ht[:], start=True, stop=True
                )
                hrow = pool.tile([1, B], F32, tag="hrow")
                nc.vector.tensor_tensor(
                    out=hrow[:], in0=ps_h[:], in1=negm_sb[0:1, a, :], op=ALU.mult
                )
                nc.vector.reduce_sum(
                    sums_sb[0:1, a, 2 * BL : 2 * BL + 1], hrow[:], axis=AX.X
                )

            nc.sync.dma_start(sums_out[:], sums_sb[:])
    nc.compile()
    return nc


def _prep_a_inputs(x, attn, q):
    """Per-core in_maps for launch A."""
    xt_full = np.ascontiguousarray(x.transpose(0, 2, 1))  # [B, D, N]
    q_t = np.ascontiguousarray(q.T).reshape(KD, 128, A)
    in_maps = []
    for c in range(NCORES):
        sl = slice(c * BL, (c + 1) * BL)
        x_nat = np.ascontiguousarray(x[sl])
        xtq = np.empty((BL, KD, 128, N + A), np.float32)
        xtq[:, :, :, 0:N] = xt_full[sl].reshape(BL, KD, 128, N)
        xtq[:, :, :, N:] = q_t[None]
        aw = np.ascontiguousarray(attn[:, sl]).reshape(L, 128, 9604)
        in_maps.append({"x_nat": x_nat, "xtq_d": xtq, "attn_w": aw})
    return in_maps


def _prep_b_inputs(pn, pos_m, neg_m):
    """pn [B, A, D] normalized pooled; masks [A, B] f32."""
    pnt = _round_f32r(
        np.ascontiguousarray(pn.transpose(1, 2, 0)).reshape(A, KD, 128, B)
    )
    # v = sum_c pn[c,a,:] * mask[a,c] -> [2, A, D]
    v = np.stack(
        [
            np.einsum("cad,ac->ad", pn.astype(np.float64), m.astype(np.float64))
            for m in (pos_m, neg_m)
        ]
    ).astype(np.float32)
    vmask = _round_f32r(
        np.ascontiguousarray(v.reshape(2, A, KD, 128).transpose(3, 0, 1, 2))
    )
    negm = np.ascontiguousarray(neg_m, dtype=np.float32).reshape(1, A, B)
    in_maps = []
    for c in range(NCORES):
        sl = slice(c * BL, (c + 1) * BL)
        pntl = np.ascontiguousarray(pnt[:, :, :, sl])
        posml = np.ascontiguousarray(pos_m.T[sl]).astype(np.float32)
        in_maps.append(
            {"pnt": pnt, "pntl": pntl, "vmask": vmask, "posml": posml, "negm": negm}
        )
    return in_maps


def kernel(common_representations, attention_weights, query_vectors, labels):
    x = np.asarray(common_representations, dtype=np.float32)
    attn = np.asarray(attention_weights, dtype=np.float32)
    q = np.asarray(query_vectors, dtype=np.float32)
    labels = np.asarray(labels)

    if "a" not in _CACHE:
        _CACHE["a"] = build_kernel_a()
    if "b" not in _CACHE:
        _CACHE["b"] = build_kernel_b()

    core_ids = list(range(NCORES))
    profile = os.environ.get("BASS_KERNEL_PROFILE", "0") == "1"
    if profile:
        _CACHE["profile"] = []
    ra = run_bass_kernel_spmd(
        _CACHE["a"], _prep_a_inputs(x, attn, q), core_ids, trace=profile
    )
    if profile:
        _CACHE["profile"].append(ra.exec_time_ns)
    res_a = ra.results

    pooled = np.concatenate([r["pooled_out"] for r in res_a], axis=0)  # [B, A, D]
    orth_rows = np.concatenate([r["orth_out"][0] for r in res_a])  # [B]
    ent = np.stack([r["ent_out"] for r in res_a])  # [NCORES, 128, 16]

    # --- host: orth + sparsity scalars ---
    orth_loss = (orth_rows.astype(np.float64).sum() - B * N) / (B * N * N)

    ent_chunks = ent.astype(np.float64).sum(axis=(0, 1))  # [16]
    per_layer = -ent_chunks.reshape(L, 4).sum(axis=1) / (B * N * N)
    lin = np.linspace(-2.0, 2.0, L)
    layer_w = 1.0 / (1.0 + np.exp(-lin))
    sparsity_loss = (layer_w * per_layer).sum() / L

    # --- host: pn + masks, then launch B ---
    pn64 = pooled.astype(np.float64)
    pn = (pn64 / np.linalg.norm(pn64, axis=-1, keepdims=True)).astype(np.float32)
    lt = labels.T  # [A, B]
    pos_m = (lt == 1).astype(np.float32)
    neg_m = (lt == 0).astype(np.float32)
    unc_m = (lt == 2).astype(np.float32)

    rb = run_bass_kernel_spmd(
        _CACHE["b"], _prep_b_inputs(pn, pos_m, neg_m), core_ids, trace=profile
    )
    if profile:
        _CACHE["profile"].append(rb.exec_time_ns)
    res_b = rb.results
    sums = np.stack([r["sums_out"][0] for r in res_b])  # [NCORES, A, 2*BL+1]

    possum = np.concatenate([sums[c, :, 0:BL] for c in range(NCORES)], axis=1)  # [A,B]
    negsum = np.concatenate([sums[c, :, BL : 2 * BL] for c in range(NCORES)], axis=1)
    hinge_sum = sums[:, :, 2 * BL].sum(axis=0)  # [A]

    n_pos = pos_m.sum(-1).astype(np.float64)
    n_neg = neg_m.sum(-1).astype(np.float64)
    n_unc = unc_m.sum(-1).astype(np.float64)
    pair_cnt = n_pos * n_neg
    pos_neg_loss = np.where(
        pair_cnt > 0, hinge_sum.astype(np.float64) / np.maximum(pair_cnt, 1.0), 0.0
    )
    pos_mean = possum.astype(np.float64) / np.maximum(n_pos, 1.0)[:, None]
    neg_mean = negsum.astype(np.float64) / np.maximum(n_neg, 1.0)[:, None]
    unc_sum = (np.abs(pos_mean - neg_mean) * unc_m).sum(axis=-1)
    unc_ok = (n_unc > 0) & (n_pos > 0) & (n_neg > 0)
    unc_loss = np.where(unc_ok, unc_sum / np.maximum(n_unc, 1.0), 0.0)
    contrastive_loss = (pos_neg_loss + unc_loss).sum() / A

    return (
        pooled.astype(np.float32),
        np.float32(orth_loss),
        np.float32(contrastive_loss),
        np.float32(sparsity_loss),
    )


# revision 22
# speedup vs baseline: 1.1259x; 1.0927x over previous
"""Trainium2 Bass kernel for AbnormalitySpecificLoss.

B=256, N=196, D=768, A=14, L=4 hardcoded; data-parallel over the batch dim
across 8 NeuronCores, two SPMD launches:

  Launch A (per core, 32-batch shard), per batch:
    - G = X X^T raw gram in two row chunks (fp32 matmuls, contraction over D).
      The second chunk's lhsT carries q^T as 14 extra stationary columns, so
      scores = q @ x^T rides along for free (psum partitions 96:110).
    - softmax on the scores rows (ACT Exp w/ accumulated Z), attn^T via PE
      transpose (tile_position=(96,0)), pooled = attn @ x (exact fp32).
    - orth partials: Q = G*G (ACT Square), s = 1/diag(G) (eye-mask + reduce +
      reciprocal), out_b = s^T Q s via tiny f32 matmuls.
    - attention-entropy partials: ACT Ln, DVE mult, DVE reduce.
  Host mid: normalize pooled -> pn, transpose to pnT, masks + v = pn^T @ mask.
  Launch B (per core): S_a = Pn_a Pn_a^T rows (f32r), hinge = pos^T relu(S-.3)
  neg via PE row-mask matmul + DVE col-mask dot; pos/neg row sums via v mm.
  Host final: counts, guards, weighted sums (exact reference formulas).
"""

import os

import numpy as np

import concourse.bacc as bacc
import concourse.mybir as mybir
import concourse.tile as tile
from concourse.bass_utils import run_bass_kernel_spmd
from concourse.masks import make_identity

# All activation funcs we use (Exp, Ln, Square, Copy, Relu) live in the
# natural_log_exp_and_others table set.  bacc's greedy per-function set choice
# would thrash between exp_and_others / natural_log every batch (~2.7us per
# reload on the ACT critical path), so empty out every other set.
_orig_gat = bacc.get_activation_tables


def _gat_one_set(arch):
    tabs = _orig_gat(arch)
    return {
        name: (fns if name == "natural_log_exp_and_others" else set())
        for name, fns in tabs.items()
    }


bacc.get_activation_tables = _gat_one_set

F32 = mybir.dt.float32
F32R = mybir.dt.float32r
AF = mybir.ActivationFunctionType
ALU = mybir.AluOpType
AX = mybir.AxisListType

B, N, D, A, L = 256, 196, 768, 14, 4
NCORES = 8
BL = B // NCORES  # 32 batches per core
KD = D // 128  # 6 contraction chunks over D
MARGIN = 0.7
R0 = 100  # gram row-chunk 0 rows; chunk 1 = N-R0 = 96 G rows + 14 score rows
R1 = N - R0  # 96 -> scores land at psum partition 96 (32-aligned for PE)

_CACHE = {}


def _round_f32r(x: np.ndarray) -> np.ndarray:
    """Round-to-nearest to 11 mantissa bits (what the PE's f32r path keeps)."""
    b = np.ascontiguousarray(x, dtype=np.float32).view(np.uint32)
    r = ((b.astype(np.uint64) + 0x800) >> 12 << 12).astype(np.uint32)
    return r.view(np.float32)


def build_kernel_a():
    nc = bacc.Bacc("TRN2", target_bir_lowering=False, debug=False, num_devices=NCORES)
    x_nat = nc.dram_tensor("x_nat", [BL, N, D], F32, kind="ExternalInput").ap()
    # xtq[b, k, p, :] = [ x^T[k*128+p, 0:196] | q^T[k*128+p, 0:14] ]
    xtq_d = nc.dram_tensor("xtq_d", [BL, KD, 128, N + A], F32, kind="ExternalInput").ap()
    attn_w = nc.dram_tensor("attn_w", [L, 128, 9604], mybir.dt.bfloat16, kind="ExternalInput").ap()

    pooled_out = nc.dram_tensor("pooled_out", [BL, A, D], F32, kind="ExternalOutput").ap()
    orth_out = nc.dram_tensor("orth_out", [1, BL], F32, kind="ExternalOutput").ap()
    ent_out = nc.dram_tensor("ent_out", [128, 16], F32, kind="ExternalOutput").ap()

    ECH = 2401  # entropy free-dim chunk (4 per layer)

    with tile.TileContext(nc) as tc:
        with (
            tc.tile_pool(name="consts", bufs=1) as consts,
            tc.tile_pool(name="sbuf", bufs=3) as pool,
            tc.tile_pool(name="psum", bufs=1, space="PSUM") as psum,
            tc.tile_pool(name="psum2", bufs=2, space="PSUM") as psum2,
        ):
            # ---- constants ----
            ident = consts.tile([128, 128], F32, tag="ident")
            make_identity(nc, ident[:])
            # eyeT[:, 0:N]: diagonal at 0 (gram chunk 0); [:, N:2N]: diag at +R0
            eyeT = consts.tile([128, 2 * N], F32, tag="eyeT")
            nc.gpsimd.memset(eyeT[:], 0.0)
            nc.gpsimd.affine_select(
                out=eyeT[:, 0:N], in_=eyeT[:, 0:N],
                compare_op=ALU.not_equal, fill=1.0,
                base=0, pattern=[[-1, N]], channel_multiplier=1,
            )
            nc.gpsimd.affine_select(
                out=eyeT[:, N : 2 * N], in_=eyeT[:, N : 2 * N],
                compare_op=ALU.not_equal, fill=1.0,
                base=R0, pattern=[[-1, N]], channel_multiplier=1,
            )
            eps_b = consts.tile([128, 1], F32, tag="eps")
            nc.vector.memset(eps_b[:], 1e-6)
            eacc = consts.tile([128, 16], F32, tag="eacc")
            orth_sb = consts.tile([1, BL], F32, tag="orth_sb")

            # ---- per-batch pipeline ----
            for b in range(BL):
                xtq = pool.tile([128, KD, N + A], F32, tag="xtq")
                nc.sync.dma_start(xtq[:], xtq_d[b].rearrange("k p n -> p k n"))
                xa = pool.tile([128, D], F32, tag="xa")
                nc.sync.dma_start(xa[:], x_nat[b, 0:128, :])
                xb = pool.tile([68, D], F32, tag="xb")
                nc.sync.dma_start(xb[:], x_nat[b, 128:N, :])

                # G row chunks; chunk1 lhsT carries q^T -> scores at rows 96:110
                ps_g0 = psum2.tile([R0, R0], F32, tag="ps_g0")
                for k in range(KD):
                    nc.tensor.matmul(
                        ps_g0[:], xtq[:, k, 0:R0], xtq[:, k, 0:R0],
                        start=(k == 0), stop=(k == KD - 1),
                    )
                ps_g1 = psum2.tile([R1 + A, N], F32, tag="ps_g1")
                for k in range(KD):
                    nc.tensor.matmul(
                        ps_g1[:], xtq[:, k, R0 : N + A], xtq[:, k, 0:N],
                        start=(k == 0), stop=(k == KD - 1),
                    )

                # softmax on scores rows [96:110] (lane-aligned slices)
                mx = pool.tile([R1 + A, 1], F32, tag="mx")
                nc.vector.reduce_max(mx[R1:, :], ps_g1[R1:, :], axis=AX.X)
                negm = pool.tile([R1 + A, 1], F32, tag="negm")
                nc.vector.tensor_scalar_mul(negm[R1:, :], mx[R1:, :], -1.0)
                expt = pool.tile([R1 + A, N], F32, tag="expt")
                zsum = pool.tile([R1 + A, 1], F32, tag="zsum")
                nc.scalar.activation(
                    expt[R1:, :], ps_g1[R1:, :], AF.Exp,
                    bias=negm[R1:, :], scale=1.0, accum_out=zsum[R1:, :],
                )
                rz = pool.tile([R1 + A, 1], F32, tag="rz")
                nc.vector.reciprocal(rz[R1:, :], zsum[R1:, :])
                attn = pool.tile([R1 + A, N], F32, tag="attn")
                nc.vector.tensor_scalar_mul(attn[R1:, :], expt[R1:, :], rz[R1:, :])

                # attn^T via PE transpose (lhsT at base partition 96)
                ps_at = psum.tile([128, 2 * A], F32, tag="ps_at")
                nc.tensor.transpose(
                    ps_at[:, 0:A], attn[R1:, 0:128],
                    ident[R1 : R1 + A, R1 : R1 + A], tile_position=(R1, 0),
                )
                nc.tensor.transpose(
                    ps_at[0:68, A : 2 * A], attn[R1:, 128:N],
                    ident[R1 : R1 + A, R1 : R1 + A], tile_position=(R1, 0),
                )
                attnT = pool.tile([128, 2 * A], F32, tag="attnT")
                nc.vector.tensor_copy(attnT[:, 0:A], ps_at[:, 0:A])
                nc.vector.tensor_copy(attnT[0:68, A : 2 * A], ps_at[0:68, A : 2 * A])

                # pooled = attn @ x -> psum [A, D] (exact fp32)
                ps_pool = psum.tile([A, D], F32, tag="ps_pool")
                for f0, f1 in ((0, 512), (512, D)):
                    nc.tensor.matmul(
                        ps_pool[:, f0:f1], attnT[:, 0:A], xa[:, f0:f1],
                        start=True, stop=False,
                    )
                    nc.tensor.matmul(
                        ps_pool[:, f0:f1], attnT[0:68, A : 2 * A], xb[:, f0:f1],
                        start=False, stop=True,
                    )
                pooled_sb = pool.tile([A, D], F32, tag="pooled_sb")
                nc.scalar.copy(pooled_sb[:], ps_pool[:])
                nc.sync.dma_start(pooled_out[b], pooled_sb[:])

                # Q = G*G via ACT Square (evacuates PSUM); diag via eye-mask
                q0 = pool.tile([R0, R0], F32, tag="q0")
                nc.scalar.activation(q0[:], ps_g0[:], AF.Square)
                q1 = pool.tile([R1, N], F32, tag="q1")
                nc.scalar.activation(q1[:], ps_g1[0:R1, :], AF.Square)
                dt0 = pool.tile([R0, R0], F32, tag="dt0")
                nc.vector.tensor_tensor(
                    out=dt0[:], in0=ps_g0[:], in1=eyeT[0:R0, 0:R0], op=ALU.mult
                )
                dt1 = pool.tile([R1, N], F32, tag="dt1")
                nc.vector.tensor_tensor(
                    out=dt1[:], in0=ps_g1[0:R1, :], in1=eyeT[0:R1, N : 2 * N],
                    op=ALU.mult,
                )
                dg0 = pool.tile([R0, 1], F32, tag="dg0")
                nc.vector.reduce_sum(dg0[:], dt0[:], axis=AX.X)
                dg1 = pool.tile([R1, 1], F32, tag="dg1")
                nc.vector.reduce_sum(dg1[:], dt1[:], axis=AX.X)
                s0 = pool.tile([R0, 1], F32, tag="s0")
                nc.vector.reciprocal(s0[:], dg0[:])
                s1 = pool.tile([R1, 1], F32, tag="s1")
                nc.vector.reciprocal(s1[:], dg1[:])

                # quadratic form via symmetry: total = S_A + 2*S_C + S_D
                ps_u = psum.tile([128, 4], F32, tag="ps_u")
                nc.tensor.matmul(ps_u[0:R0, 0:1], q0[:], s0[:], start=True, stop=True)
                nc.tensor.matmul(ps_u[0:R0, 1:2], q1[:, 0:R0], s1[:], start=True, stop=True)
                nc.tensor.matmul(ps_u[0:R1, 2:3], q1[:, R0:N], s1[:], start=True, stop=True)
                uA = pool.tile([R0, 1], F32, tag="uA")
                nc.vector.tensor_copy(uA[:], ps_u[0:R0, 0:1])
                v1a = pool.tile([R0, 1], F32, tag="v1a")
                nc.vector.tensor_copy(v1a[:], ps_u[0:R0, 1:2])
                v1b = pool.tile([R1, 1], F32, tag="v1b")
                nc.vector.tensor_copy(v1b[:], ps_u[0:R1, 2:3])
                nc.tensor.matmul(ps_u[0:1, 3:4], uA[:], s0[:], start=True, stop=False)
                nc.tensor.matmul(ps_u[0:1, 3:4], v1b[:], s1[:], start=False, stop=False)
                nc.tensor.matmul(ps_u[0:1, 3:4], v1a[:], s0[:], start=False, stop=False)
                nc.tensor.matmul(ps_u[0:1, 3:4], v1a[:], s0[:], start=False, stop=True)
                nc.vector.tensor_copy(orth_sb[0:1, b : b + 1], ps_u[0:1, 3:4])

                # entropy chunk (one per two batches)
                if b % 2 == 0:
                    c = b // 2
                    lay, sub = c // 4, c % 4
                    wt = pool.tile([128, ECH], mybir.dt.bfloat16, tag="wt")
                    nc.sync.dma_start(
                        wt[:], attn_w[lay, :, sub * ECH : (sub + 1) * ECH]
                    )
                    lnt = pool.tile([128, ECH], F32, tag="lnt")
                    nc.scalar.activation(lnt[:], wt[:], AF.Ln, bias=eps_b[:], scale=1.0)
                    prod = pool.tile([128, ECH], F32, tag="prod")
                    nc.vector.tensor_tensor(out=prod[:], in0=wt[:], in1=lnt[:], op=ALU.mult)
                    dump = pool.tile([128, ECH], F32, tag="dump")
                    nc.scalar.activation(
                        dump[:], prod[:], AF.Copy, accum_out=eacc[:, c : c + 1]
                    )

            nc.sync.dma_start(orth_out[:], orth_sb[:])
            nc.sync.dma_start(ent_out[:], eacc[:])
    nc.compile()
    return nc


def build_kernel_b():
    nc = bacc.Bacc("TRN2", target_bir_lowering=False, debug=False, num_devices=NCORES)
    # replicated: pnT [A, D, B] (f32r), v masks [2, A, KD, 128] (f32r), negm rows
    pnt = nc.dram_tensor("pnt", [A, KD, 128, B], F32R, kind="ExternalInput").ap()
    # per-core: local columns slice [A, KD, 128, BL], local pos mask [BL, A]
    pntl = nc.dram_tensor("pntl", [A, KD, 128, BL], F32R, kind="ExternalInput").ap()
    vmask = nc.dram_tensor("vmask", [128, 2, A, KD], F32R, kind="ExternalInput").ap()
    posml = nc.dram_tensor("posml", [BL, A], F32, kind="ExternalInput").ap()
    negm = nc.dram_tensor("negm", [1, A, B], F32, kind="ExternalInput").ap()

    # sums[0, a, 0:B?]: possum/negsum per (a, local b) plus hinge value
    sums_out = nc.dram_tensor("sums_out", [1, A, 2 * BL + 1], F32, kind="ExternalOutput").ap()

    with tile.TileContext(nc) as tc:
        with (
            tc.tile_pool(name="consts", bufs=1) as consts,
            tc.tile_pool(name="sbuf", bufs=4) as pool,
            tc.tile_pool(name="psum", bufs=2, space="PSUM") as psum,
        ):
            posm_sb = consts.tile([BL, A], F32, tag="posm")
            nc.sync.dma_start(posm_sb[:], posml[:])
            negm_sb = consts.tile([1, A, B], F32, tag="negm")
            nc.sync.dma_start(negm_sb[:], negm[:])
            vm_sb = consts.tile([128, 2, A, KD], F32R, tag="vm")
            nc.sync.dma_start(vm_sb[:], vmask[:])
            mb = consts.tile([BL, 1], F32, tag="mb")
            nc.vector.memset(mb[:], MARGIN - 1.0)
            sums_sb = consts.tile([1, A, 2 * BL + 1], F32, tag="sums")

            for a in range(A):
                pt = pool.tile([128, KD, B], F32R, tag="pt")
                nc.sync.dma_start(pt[:], pnt[a].rearrange("k p c -> p k c"))
                ptl = pool.tile([128, KD, BL], F32R, tag="ptl")
                nc.sync.dma_start(ptl[:], pntl[a].rearrange("k p c -> p k c"))

                # S rows: [BL, B]
                ps_s = psum.tile([BL, B], F32, tag="ps_s")
                for k in range(KD):
                    nc.tensor.matmul(
                        ps_s[:], ptl[:, k, :], pt[:, k, :],
                        start=(k == 0), stop=(k == KD - 1),
                    )
                # possum/negsum: v^T @ pnT_local -> [1, BL] each
                ps_vs = psum.tile([1, 2 * BL], F32, tag="ps_vs")
                for m in range(2):
                    for k in range(KD):
                        nc.tensor.matmul(
                            ps_vs[:, m * BL : (m + 1) * BL],
                            vm_sb[:, m, a, k : k + 1],
                            ptl[:, k, :],
                            start=(k == 0), stop=(k == KD - 1),
                        )
                nc.vector.tensor_copy(sums_sb[0:1, a, 0 : 2 * BL], ps_vs[:])

                # hinge: H = relu(S - 0.3); col = posm^T @ H; dot with negm
                ht = pool.tile([BL, B], F32, tag="ht")
                nc.scalar.activation(ht[:], ps_s[:], AF.Relu, bias=mb[:], scale=1.0)
                ps_h = psum.tile([1, B], F32, tag="ps_h")
                nc.tensor.matmul(
                    ps_h[:], posm_sb[:, a : a + 1], ht[:], start=True, stop=True
                )
                hrow = pool.tile([1, B], F32, tag="hrow")
                nc.vector.tensor_tensor(
                    out=hrow[:], in0=ps_h[:], in1=negm_sb[0:1, a, :], op=ALU.mult
                )
                nc.vector.reduce_sum(
                    sums_sb[0:1, a, 2 * BL : 2 * BL + 1], hrow[:], axis=AX.X
                )

            nc.sync.dma_start(sums_out[:], sums_sb[:])
    nc.compile()
    return nc


def _prep_a_inputs(x, attn, q):
    """Per-core in_maps for launch A."""
    xt_full = np.ascontiguousarray(x.transpose(0, 2, 1))  # [B, D, N]
    q_t = np.ascontiguousarray(q.T).reshape(KD, 128, A)
    in_maps = []
    for c in range(NCORES):
        sl = slice(c * BL, (c + 1) * BL)
        x_nat = np.ascontiguousarray(x[sl])
        xtq = np.empty((BL, KD, 128, N + A), np.float32)
        xtq[:, :, :, 0:N] = xt_full[sl].reshape(BL, KD, 128, N)
        xtq[:, :, :, N:] = q_t[None]
        import ml_dtypes
        aw = np.ascontiguousarray(attn[:, sl]).reshape(L, 128, 9604)
        aw = aw.astype(ml_dtypes.bfloat16)
        in_maps.append({"x_nat": x_nat, "xtq_d": xtq, "attn_w": aw})
    return in_maps


def _prep_b_inputs(pn, pos_m, neg_m):
    """pn [B, A, D] normalized pooled; masks [A, B] f32."""
    pnt = _round_f32r(
        np.ascontiguousarray(pn.transpose(1, 2, 0)).reshape(A, KD, 128, B)
    )
    # v = sum_c pn[c,a,:] * mask[a,c] -> [2, A, D]
    v = np.stack(
        [
            np.einsum("cad,ac->ad", pn.astype(np.float64), m.astype(np.float64))
            for m in (pos_m, neg_m)
        ]
    ).astype(np.float32)
    vmask = _round_f32r(
        np.ascontiguousarray(v.reshape(2, A, KD, 128).transpose(3, 0, 1, 2))
    )
    negm = np.ascontiguousarray(neg_m, dtype=np.float32).reshape(1, A, B)
    in_maps = []
    for c in range(NCORES):
        sl = slice(c * BL, (c + 1) * BL)
        pntl = np.ascontiguousarray(pnt[:, :, :, sl])
        posml = np.ascontiguousarray(pos_m.T[sl]).astype(np.float32)
        in_maps.append(
            {"pnt": pnt, "pntl": pntl, "vmask": vmask, "posml": posml, "negm": negm}
        )
    return in_maps


def kernel(common_representations, attention_weights, query_vectors, labels):
    x = np.asarray(common_representations, dtype=np.float32)
    attn = np.asarray(attention_weights, dtype=np.float32)
    q = np.asarray(query_vectors, dtype=np.float32)
    labels = np.asarray(labels)

    if "a" not in _CACHE:
        _CACHE["a"] = build_kernel_a()
    if "b" not in _CACHE:
        _CACHE["b"] = build_kernel_b()

    core_ids = list(range(NCORES))
    profile = os.environ.get("BASS_KERNEL_PROFILE", "0") == "1"
    if profile:
        _CACHE["profile"] = []
    ra = run_bass_kernel_spmd(
        _CACHE["a"], _prep_a_inputs(x, attn, q), core_ids, trace=profile
    )
    if profile:
        _CACHE["profile"].append(ra.exec_time_ns)
    res_a = ra.results

    pooled = np.concatenate([r["pooled_out"] for r in res_a], axis=0)  # [B, A, D]
    orth_rows = np.concatenate([r["orth_out"][0] for r in res_a])  # [B]
    ent = np.stack([r["ent_out"] for r in res_a])  # [NCORES, 128, 16]

    # --- host: orth + sparsity scalars ---
    orth_loss = (orth_rows.astype(np.float64).sum() - B * N) / (B * N * N)

    ent_chunks = ent.astype(np.float64).sum(axis=(0, 1))  # [16]
    per_layer = -ent_chunks.reshape(L, 4).sum(axis=1) / (B * N * N)
    lin = np.linspace(-2.0, 2.0, L)
    layer_w = 1.0 / (1.0 + np.exp(-lin))
    sparsity_loss = (layer_w * per_layer).sum() / L

    # --- host: pn + masks, then launch B ---
    pn64 = pooled.astype(np.float64)
    pn = (pn64 / np.linalg.norm(pn64, axis=-1, keepdims=True)).astype(np.float32)
    lt = labels.T  # [A, B]
    pos_m = (lt == 1).astype(np.float32)
    neg_m = (lt == 0).astype(np.float32)
    unc_m = (lt == 2).astype(np.float32)

    rb = run_bass_kernel_spmd(
        _CACHE["b"], _prep_b_inputs(pn, pos_m, neg_m), core_ids, trace=profile
    )
    if profile:
        _CACHE["profile"].append(rb.exec_time_ns)
    res_b = rb.results
    sums = np.stack([r["sums_out"][0] for r in res_b])  # [NCORES, A, 2*BL+1]

    possum = np.concatenate([sums[c, :, 0:BL] for c in range(NCORES)], axis=1)  # [A,B]
    negsum = np.concatenate([sums[c, :, BL : 2 * BL] for c in range(NCORES)], axis=1)
    hinge_sum = sums[:, :, 2 * BL].sum(axis=0)  # [A]

    n_pos = pos_m.sum(-1).astype(np.float64)
    n_neg = neg_m.sum(-1).astype(np.float64)
    n_unc = unc_m.sum(-1).astype(np.float64)
    pair_cnt = n_pos * n_neg
    pos_neg_loss = np.where(
        pair_cnt > 0, hinge_sum.astype(np.float64) / np.maximum(pair_cnt, 1.0), 0.0
    )
    pos_mean = possum.astype(np.float64) / np.maximum(n_pos, 1.0)[:, None]
    neg_mean = negsum.astype(np.float64) / np.maximum(n_neg, 1.0)[:, None]
    unc_sum = (np.abs(pos_mean - neg_mean) * unc_m).sum(axis=-1)
    unc_ok = (n_unc > 0) & (n_pos > 0) & (n_neg > 0)
    unc_loss = np.where(unc_ok, unc_sum / np.maximum(n_unc, 1.0), 0.0)
    contrastive_loss = (pos_neg_loss + unc_loss).sum() / A

    return (
        pooled.astype(np.float32),
        np.float32(orth_loss),
        np.float32(contrastive_loss),
        np.float32(sparsity_loss),
    )


# revision 23
# speedup vs baseline: 1.1556x; 1.0264x over previous
"""Trainium2 Bass kernel for AbnormalitySpecificLoss.

B=256, N=196, D=768, A=14, L=4 hardcoded; data-parallel over the batch dim
across 8 NeuronCores, two SPMD launches:

  Launch A (per core, 32-batch shard), per batch:
    - G = X X^T raw gram in two row chunks (fp32 matmuls, contraction over D).
      The second chunk's lhsT carries q^T as 14 extra stationary columns, so
      scores = q @ x^T rides along for free (psum partitions 96:110).
    - softmax on the scores rows (ACT Exp w/ accumulated Z), attn^T via PE
      transpose (tile_position=(96,0)), pooled = attn @ x (exact fp32).
    - orth partials: Q = G*G (ACT Square), s = 1/diag(G) (eye-mask + reduce +
      reciprocal), out_b = s^T Q s via tiny f32 matmuls.
    - attention-entropy partials: ACT Ln, DVE mult, DVE reduce.
  Host mid: normalize pooled -> pn, transpose to pnT, masks + v = pn^T @ mask.
  Launch B (per core): S_a = Pn_a Pn_a^T rows (f32r), hinge = pos^T relu(S-.3)
  neg via PE row-mask matmul + DVE col-mask dot; pos/neg row sums via v mm.
  Host final: counts, guards, weighted sums (exact reference formulas).
"""

import os

import numpy as np

import concourse.bacc as bacc
import concourse.mybir as mybir
import concourse.tile as tile
from concourse.bass_utils import run_bass_kernel_spmd
from concourse.masks import make_identity

# All activation funcs we use (Exp, Ln, Square, Copy, Relu) live in the
# natural_log_exp_and_others table set.  bacc's greedy per-function set choice
# would thrash between exp_and_others / natural_log every batch (~2.7us per
# reload on the ACT critical path), so empty out every other set.
_orig_gat = bacc.get_activation_tables


def _gat_one_set(arch):
    tabs = _orig_gat(arch)
    return {
        name: (fns if name == "natural_log_exp_and_others" else set())
        for name, fns in tabs.items()
    }


bacc.get_activation_tables = _gat_one_set

F32 = mybir.dt.float32
F32R = mybir.dt.float32r
AF = mybir.ActivationFunctionType
ALU = mybir.AluOpType
AX = mybir.AxisListType

B, N, D, A, L = 256, 196, 768, 14, 4
NCORES = 8
BL = B // NCORES  # 32 batches per core
KD = D // 128  # 6 contraction chunks over D
MARGIN = 0.7
R0 = 100  # gram row-chunk 0 rows; chunk 1 = N-R0 = 96 G rows + 14 score rows
R1 = N - R0  # 96 -> scores land at psum partition 96 (32-aligned for PE)

_CACHE = {}


def _round_f32r(x: np.ndarray) -> np.ndarray:
    """Round-to-nearest to 11 mantissa bits (what the PE's f32r path keeps)."""
    b = np.ascontiguousarray(x, dtype=np.float32).view(np.uint32)
    r = ((b.astype(np.uint64) + 0x800) >> 12 << 12).astype(np.uint32)
    return r.view(np.float32)


def build_kernel_a():
    nc = bacc.Bacc("TRN2", target_bir_lowering=False, debug=False, num_devices=NCORES)
    x_nat = nc.dram_tensor("x_nat", [BL, N, D], F32, kind="ExternalInput").ap()
    # xtq[b, k, p, :] = [ x^T[k*128+p, 0:196] | q^T[k*128+p, 0:14] ]
    xtq_d = nc.dram_tensor("xtq_d", [BL, KD, 128, N + A], F32, kind="ExternalInput").ap()
    attn_w = nc.dram_tensor("attn_w", [L, 128, 9604], mybir.dt.bfloat16, kind="ExternalInput").ap()

    pooled_out = nc.dram_tensor("pooled_out", [BL, A, D], F32, kind="ExternalOutput").ap()
    orth_out = nc.dram_tensor("orth_out", [1, BL], F32, kind="ExternalOutput").ap()
    ent_out = nc.dram_tensor("ent_out", [128, 16], F32, kind="ExternalOutput").ap()

    ECH = 2401  # entropy free-dim chunk (4 per layer)

    with tile.TileContext(nc) as tc:
        with (
            tc.tile_pool(name="consts", bufs=1) as consts,
            tc.tile_pool(name="sbuf", bufs=3) as pool,
            tc.tile_pool(name="psum", bufs=1, space="PSUM") as psum,
            tc.tile_pool(name="psum2", bufs=2, space="PSUM") as psum2,
        ):
            # ---- constants ----
            ident = consts.tile([128, 128], F32, tag="ident")
            make_identity(nc, ident[:])
            # eyeT[:, 0:N]: diagonal at 0 (gram chunk 0); [:, N:2N]: diag at +R0
            eyeT = consts.tile([128, 2 * N], F32, tag="eyeT")
            nc.gpsimd.memset(eyeT[:], 0.0)
            nc.gpsimd.affine_select(
                out=eyeT[:, 0:N], in_=eyeT[:, 0:N],
                compare_op=ALU.not_equal, fill=1.0,
                base=0, pattern=[[-1, N]], channel_multiplier=1,
            )
            nc.gpsimd.affine_select(
                out=eyeT[:, N : 2 * N], in_=eyeT[:, N : 2 * N],
                compare_op=ALU.not_equal, fill=1.0,
                base=R0, pattern=[[-1, N]], channel_multiplier=1,
            )
            eps_b = consts.tile([128, 1], F32, tag="eps")
            nc.vector.memset(eps_b[:], 1e-6)
            eacc = consts.tile([128, 16], F32, tag="eacc")
            orth_sb = consts.tile([1, BL], F32, tag="orth_sb")

            # ---- per-batch pipeline ----
            for b in range(BL):
                xtq = pool.tile([128, KD, N + A], F32, tag="xtq")
                nc.sync.dma_start(xtq[:], xtq_d[b].rearrange("k p n -> p k n"))
                xa = pool.tile([128, D], F32, tag="xa")
                nc.sync.dma_start(xa[:], x_nat[b, 0:128, :])
                xb = pool.tile([68, D], F32, tag="xb")
                nc.sync.dma_start(xb[:], x_nat[b, 128:N, :])

                # G row chunks; chunk1 lhsT carries q^T -> scores at rows 96:110.
                # G0 feeds only orth statistics -> bf16 (1cyc/row) via a small
                # on-device cast that overlaps with the fp32 G1 matmuls.
                BF16 = mybir.dt.bfloat16
                xbf = pool.tile([128, KD, R0], BF16, tag="xbf")
                nc.vector.tensor_copy(xbf[:], xtq[:, :, 0:R0])
                ps_g1 = psum2.tile([R1 + A, N], F32, tag="ps_g1")
                for k in range(KD):
                    nc.tensor.matmul(
                        ps_g1[:], xtq[:, k, R0 : N + A], xtq[:, k, 0:N],
                        start=(k == 0), stop=(k == KD - 1),
                    )
                ps_g0 = psum2.tile([R0, R0], F32, tag="ps_g0")
                for k in range(KD):
                    nc.tensor.matmul(
                        ps_g0[:], xbf[:, k, :], xbf[:, k, :],
                        start=(k == 0), stop=(k == KD - 1),
                    )

                # softmax on scores rows [96:110] (lane-aligned slices)
                mx = pool.tile([R1 + A, 1], F32, tag="mx")
                nc.vector.reduce_max(mx[R1:, :], ps_g1[R1:, :], axis=AX.X)
                negm = pool.tile([R1 + A, 1], F32, tag="negm")
                nc.vector.tensor_scalar_mul(negm[R1:, :], mx[R1:, :], -1.0)
                expt = pool.tile([R1 + A, N], F32, tag="expt")
                zsum = pool.tile([R1 + A, 1], F32, tag="zsum")
                nc.scalar.activation(
                    expt[R1:, :], ps_g1[R1:, :], AF.Exp,
                    bias=negm[R1:, :], scale=1.0, accum_out=zsum[R1:, :],
                )
                rz = pool.tile([R1 + A, 1], F32, tag="rz")
                nc.vector.reciprocal(rz[R1:, :], zsum[R1:, :])
                attn = pool.tile([R1 + A, N], F32, tag="attn")
                nc.vector.tensor_scalar_mul(attn[R1:, :], expt[R1:, :], rz[R1:, :])

                # attn^T via PE transpose (lhsT at base partition 96)
                ps_at = psum.tile([128, 2 * A], F32, tag="ps_at")
                nc.tensor.transpose(
                    ps_at[:, 0:A], attn[R1:, 0:128],
                    ident[R1 : R1 + A, R1 : R1 + A], tile_position=(R1, 0),
                )
                nc.tensor.transpose(
                    ps_at[0:68, A : 2 * A], attn[R1:, 128:N],
                    ident[R1 : R1 + A, R1 : R1 + A], tile_position=(R1, 0),
                )
                attnT = pool.tile([128, 2 * A], F32, tag="attnT")
                nc.vector.tensor_copy(attnT[:, 0:A], ps_at[:, 0:A])
                nc.vector.tensor_copy(attnT[0:68, A : 2 * A], ps_at[0:68, A : 2 * A])

                # pooled = attn @ x -> psum [A, D] (exact fp32)
                ps_pool = psum.tile([A, D], F32, tag="ps_pool")
                for f0, f1 in ((0, 512), (512, D)):
                    nc.tensor.matmul(
                        ps_pool[:, f0:f1], attnT[:, 0:A], xa[:, f0:f1],
                        start=True, stop=False,
                    )
                    nc.tensor.matmul(
                        ps_pool[:, f0:f1], attnT[0:68, A : 2 * A], xb[:, f0:f1],
                        start=False, stop=True,
                    )
                pooled_sb = pool.tile([A, D], F32, tag="pooled_sb")
                nc.scalar.copy(pooled_sb[:], ps_pool[:])
                nc.sync.dma_start(pooled_out[b], pooled_sb[:])

                # Q = G*G via ACT Square (evacuates PSUM); diag via eye-mask
                q0 = pool.tile([R0, R0], F32, tag="q0")
                nc.scalar.activation(q0[:], ps_g0[:], AF.Square)
                q1 = pool.tile([R1, N], F32, tag="q1")
                nc.scalar.activation(q1[:], ps_g1[0:R1, :], AF.Square)
                dt0 = pool.tile([R0, R0], F32, tag="dt0")
                nc.vector.tensor_tensor(
                    out=dt0[:], in0=ps_g0[:], in1=eyeT[0:R0, 0:R0], op=ALU.mult
                )
                dt1 = pool.tile([R1, N], F32, tag="dt1")
                nc.vector.tensor_tensor(
                    out=dt1[:], in0=ps_g1[0:R1, :], in1=eyeT[0:R1, N : 2 * N],
                    op=ALU.mult,
                )
                dg0 = pool.tile([R0, 1], F32, tag="dg0")
                nc.vector.reduce_sum(dg0[:], dt0[:], axis=AX.X)
                dg1 = pool.tile([R1, 1], F32, tag="dg1")
                nc.vector.reduce_sum(dg1[:], dt1[:], axis=AX.X)
                s0 = pool.tile([R0, 1], F32, tag="s0")
                nc.vector.reciprocal(s0[:], dg0[:])
                s1 = pool.tile([R1, 1], F32, tag="s1")
                nc.vector.reciprocal(s1[:], dg1[:])

                # quadratic form via symmetry: total = S_A + 2*S_C + S_D
                ps_u = psum.tile([128, 4], F32, tag="ps_u")
                nc.tensor.matmul(ps_u[0:R0, 0:1], q0[:], s0[:], start=True, stop=True)
                nc.tensor.matmul(ps_u[0:R0, 1:2], q1[:, 0:R0], s1[:], start=True, stop=True)
                nc.tensor.matmul(ps_u[0:R1, 2:3], q1[:, R0:N], s1[:], start=True, stop=True)
                uA = pool.tile([R0, 1], F32, tag="uA")
                nc.vector.tensor_copy(uA[:], ps_u[0:R0, 0:1])
                v1a = pool.tile([R0, 1], F32, tag="v1a")
                nc.vector.tensor_copy(v1a[:], ps_u[0:R0, 1:2])
                v1b = pool.tile([R1, 1], F32, tag="v1b")
                nc.vector.tensor_copy(v1b[:], ps_u[0:R1, 2:3])
                nc.tensor.matmul(ps_u[0:1, 3:4], uA[:], s0[:], start=True, stop=False)
                nc.tensor.matmul(ps_u[0:1, 3:4], v1b[:], s1[:], start=False, stop=False)
                nc.tensor.matmul(ps_u[0:1, 3:4], v1a[:], s0[:], start=False, stop=False)
                nc.tensor.matmul(ps_u[0:1, 3:4], v1a[:], s0[:], start=False, stop=True)
                nc.vector.tensor_copy(orth_sb[0:1, b : b + 1], ps_u[0:1, 3:4])

                # entropy chunk (one per two batches)
                if b % 2 == 0:
                    c = b // 2
                    lay, sub = c // 4, c % 4
                    wt = pool.tile([128, ECH], mybir.dt.bfloat16, tag="wt")
                    nc.sync.dma_start(
                        wt[:], attn_w[lay, :, sub * ECH : (sub + 1) * ECH]
                    )
                    lnt = pool.tile([128, ECH], F32, tag="lnt")
                    nc.scalar.activation(lnt[:], wt[:], AF.Ln, bias=eps_b[:], scale=1.0)
                    prod = pool.tile([128, ECH], F32, tag="prod")
                    nc.vector.tensor_tensor(out=prod[:], in0=wt[:], in1=lnt[:], op=ALU.mult)
                    dump = pool.tile([128, ECH], F32, tag="dump")
                    nc.scalar.activation(
                        dump[:], prod[:], AF.Copy, accum_out=eacc[:, c : c + 1]
                    )

            nc.sync.dma_start(orth_out[:], orth_sb[:])
            nc.sync.dma_start(ent_out[:], eacc[:])
    nc.compile()
    return nc


def build_kernel_b():
    nc = bacc.Bacc("TRN2", target_bir_lowering=False, debug=False, num_devices=NCORES)
    # replicated: pnT [A, D, B] (f32r), v masks [2, A, KD, 128] (f32r), negm rows
    pnt = nc.dram_tensor("pnt", [A, KD, 128, B], F32R, kind="ExternalInput").ap()
    # per-core: local columns slice [A, KD, 128, BL], local pos mask [BL, A]
    pntl = nc.dram_tensor("pntl", [A, KD, 128, BL], F32R, kind="ExternalInput").ap()
    vmask = nc.dram_tensor("vmask", [128, 2, A, KD], F32R, kind="ExternalInput").ap()
    posml = nc.dram_tensor("posml", [BL, A], F32, kind="ExternalInput").ap()
    negm = nc.dram_tensor("negm", [1, A, B], F32, kind="ExternalInput").ap()

    # sums[0, a, 0:B?]: possum/negsum per (a, local b) plus hinge value
    sums_out = nc.dram_tensor("sums_out", [1, A, 2 * BL + 1], F32, kind="ExternalOutput").ap()

    with tile.TileContext(nc) as tc:
        with (
            tc.tile_pool(name="consts", bufs=1) as consts,
            tc.tile_pool(name="sbuf", bufs=4) as pool,
            tc.tile_pool(name="psum", bufs=2, space="PSUM") as psum,
        ):
            posm_sb = consts.tile([BL, A], F32, tag="posm")
            nc.sync.dma_start(posm_sb[:], posml[:])
            negm_sb = consts.tile([1, A, B], F32, tag="negm")
            nc.sync.dma_start(negm_sb[:], negm[:])
            vm_sb = consts.tile([128, 2, A, KD], F32R, tag="vm")
            nc.sync.dma_start(vm_sb[:], vmask[:])
            mb = consts.tile([BL, 1], F32, tag="mb")
            nc.vector.memset(mb[:], MARGIN - 1.0)
            sums_sb = consts.tile([1, A, 2 * BL + 1], F32, tag="sums")

            for a in range(A):
                pt = pool.tile([128, KD, B], F32R, tag="pt")
                nc.sync.dma_start(pt[:], pnt[a].rearrange("k p c -> p k c"))
                ptl = pool.tile([128, KD, BL], F32R, tag="ptl")
                nc.sync.dma_start(ptl[:], pntl[a].rearrange("k p c -> p k c"))

                # S rows: [BL, B]
                ps_s = psum.tile([BL, B], F32, tag="ps_s")
                for k in range(KD):
                    nc.tensor.matmul(
                        ps_s[:], ptl[:, k, :], pt[:, k, :],
                        start=(k == 0), stop=(k == KD - 1),
                    )
                # possum/negsum: v^T @ pnT_local -> [1, BL] each
                ps_vs = psum.tile([1, 2 * BL], F32, tag="ps_vs")
                for m in range(2):
                    for k in range(KD):
                        nc.tensor.matmul(
                            ps_vs[:, m * BL : (m + 1) * BL],
                            vm_sb[:, m, a, k : k + 1],
                            ptl[:, k, :],
                            start=(k == 0), stop=(k == KD - 1),
                        )
                nc.vector.tensor_copy(sums_sb[0:1, a, 0 : 2 * BL], ps_vs[:])

                # hinge: H = relu(S - 0.3); col = posm^T @ H; dot with negm
                ht = pool.tile([BL, B], F32, tag="ht")
                nc.scalar.activation(ht[:], ps_s[:], AF.Relu, bias=mb[:], scale=1.0)
                ps_h = psum.tile([1, B], F32, tag="ps_h")
                nc.tensor.matmul(
                    ps_h[:], posm_sb[:, a : a + 1], ht[:], start=True, stop=True
                )
                hrow = pool.tile([1, B], F32, tag="hrow")
                nc.vector.tensor_tensor(
                    out=hrow[:], in0=ps_h[:], in1=negm_sb[0:1, a, :], op=ALU.mult
                )
                nc.vector.reduce_sum(
                    sums_sb[0:1, a, 2 * BL : 2 * BL + 1], hrow[:], axis=AX.X
                )

            nc.sync.dma_start(sums_out[:], sums_sb[:])
    nc.compile()
    return nc


def _prep_a_inputs(x, attn, q):
    """Per-core in_maps for launch A."""
    xt_full = np.ascontiguousarray(x.transpose(0, 2, 1))  # [B, D, N]
    q_t = np.ascontiguousarray(q.T).reshape(KD, 128, A)
    in_maps = []
    for c in range(NCORES):
        sl = slice(c * BL, (c + 1) * BL)
        x_nat = np.ascontiguousarray(x[sl])
        xtq = np.empty((BL, KD, 128, N + A), np.float32)
        xtq[:, :, :, 0:N] = xt_full[sl].reshape(BL, KD, 128, N)
        xtq[:, :, :, N:] = q_t[None]
        import ml_dtypes
        aw = np.ascontiguousarray(attn[:, sl]).reshape(L, 128, 9604)
        aw = aw.astype(ml_dtypes.bfloat16)
        in_maps.append({"x_nat": x_nat, "xtq_d": xtq, "attn_w": aw})
    return in_maps


def _prep_b_inputs(pn, pos_m, neg_m):
    """pn [B, A, D] normalized pooled; masks [A, B] f32."""
    pnt = _round_f32r(
        np.ascontiguousarray(pn.transpose(1, 2, 0)).reshape(A, KD, 128, B)
    )
    # v = sum_c pn[c,a,:] * mask[a,c] -> [2, A, D]
    v = np.stack(
        [
            np.einsum("cad,ac->ad", pn.astype(np.float64), m.astype(np.float64))
            for m in (pos_m, neg_m)
        ]
    ).astype(np.float32)
    vmask = _round_f32r(
        np.ascontiguousarray(v.reshape(2, A, KD, 128).transpose(3, 0, 1, 2))
    )
    negm = np.ascontiguousarray(neg_m, dtype=np.float32).reshape(1, A, B)
    in_maps = []
    for c in range(NCORES):
        sl = slice(c * BL, (c + 1) * BL)
        pntl = np.ascontiguousarray(pnt[:, :, :, sl])
        posml = np.ascontiguousarray(pos_m.T[sl]).astype(np.float32)
        in_maps.append(
            {"pnt": pnt, "pntl": pntl, "vmask": vmask, "posml": posml, "negm": negm}
        )
    return in_maps


def kernel(common_representations, attention_weights, query_vectors, labels):
    x = np.asarray(common_representations, dtype=np.float32)
    attn = np.asarray(attention_weights, dtype=np.float32)
    q = np.asarray(query_vectors, dtype=np.float32)
    labels = np.asarray(labels)

    if "a" not in _CACHE:
        _CACHE["a"] = build_kernel_a()
    if "b" not in _CACHE:
        _CACHE["b"] = build_kernel_b()

    core_ids = list(range(NCORES))
    profile = os.environ.get("BASS_KERNEL_PROFILE", "0") == "1"
    if profile:
        _CACHE["profile"] = []
    ra = run_bass_kernel_spmd(
        _CACHE["a"], _prep_a_inputs(x, attn, q), core_ids, trace=profile
    )
    if profile:
        _CACHE["profile"].append(ra.exec_time_ns)
    res_a = ra.results

    pooled = np.concatenate([r["pooled_out"] for r in res_a], axis=0)  # [B, A, D]
    orth_rows = np.concatenate([r["orth_out"][0] for r in res_a])  # [B]
    ent = np.stack([r["ent_out"] for r in res_a])  # [NCORES, 128, 16]

    # --- host: orth + sparsity scalars ---
    orth_loss = (orth_rows.astype(np.float64).sum() - B * N) / (B * N * N)

    ent_chunks = ent.astype(np.float64).sum(axis=(0, 1))  # [16]
    per_layer = -ent_chunks.reshape(L, 4).sum(axis=1) / (B * N * N)
    lin = np.linspace(-2.0, 2.0, L)
    layer_w = 1.0 / (1.0 + np.exp(-lin))
    sparsity_loss = (layer_w * per_layer).sum() / L

    # --- host: pn + masks, then launch B ---
    pn64 = pooled.astype(np.float64)
    pn = (pn64 / np.linalg.norm(pn64, axis=-1, keepdims=True)).astype(np.float32)
    lt = labels.T  # [A, B]
    pos_m = (lt == 1).astype(np.float32)
    neg_m = (lt == 0).astype(np.float32)
    unc_m = (lt == 2).astype(np.float32)

    rb = run_bass_kernel_spmd(
        _CACHE["b"], _prep_b_inputs(pn, pos_m, neg_m), core_ids, trace=profile
    )
    if profile:
        _CACHE["profile"].append(rb.exec_time_ns)
    res_b = rb.results
    sums = np.stack([r["sums_out"][0] for r in res_b])  # [NCORES, A, 2*BL+1]

    possum = np.concatenate([sums[c, :, 0:BL] for c in range(NCORES)], axis=1)  # [A,B]
    negsum = np.concatenate([sums[c, :, BL : 2 * BL] for c in range(NCORES)], axis=1)
    hinge_sum = sums[:, :, 2 * BL].sum(axis=0)  # [A]

    n_pos = pos_m.sum(-1).astype(np.float64)
    n_neg = neg_m.sum(-1).astype(np.float64)
    n_unc = unc_m.sum(-1).astype(np.float64)
    pair_cnt = n_pos * n_neg
    pos_neg_loss = np.where(
        pair_cnt > 0, hinge_sum.astype(np.float64) / np.maximum(pair_cnt, 1.0), 0.0
    )
    pos_mean = possum.astype(np.float64) / np.maximum(n_pos, 1.0)[:, None]
    neg_mean = negsum.astype(np.float64) / np.maximum(n_neg, 1.0)[:, None]
    unc_sum = (np.abs(pos_mean - neg_mean) * unc_m).sum(axis=-1)
    unc_ok = (n_unc > 0) & (n_pos > 0) & (n_neg > 0)
    unc_loss = np.where(unc_ok, unc_sum / np.maximum(n_unc, 1.0), 0.0)
    contrastive_loss = (pos_neg_loss + unc_loss).sum() / A

    return (
        pooled.astype(np.float32),
        np.float32(orth_loss),
        np.float32(contrastive_loss),
        np.float32(sparsity_loss),
    )
